# revision 3
# baseline (speedup 1.0000x reference)
"""ANI-style element-MLP (MoE routing) kernel for 8 TRN2 NeuronCores.

Strategy:
  - Host: bucket atoms by element (expert). Only ~4/9 of atoms match any
    expert; the rest contribute 0.  Each expert bucket is padded to a fixed
    capacity, split in half, and each half is assigned to one core
    (cores 2e, 2e+1 own expert e).  Per-core inputs are the gathered,
    transposed representation rows [D, S] plus that expert's weights laid
    out in SBUF-ready [128, ...] chunk order.
  - Device: 3-layer MLP as tiled matmuls (features on partitions so biases
    are per-partition ACT bias), softplus on the scalar engine.  The
    softplus -log(2) shift is folded into the next layer's bias on host.
    Output is the per-slot scalar energy [1, S] per core.
  - Host: scatter-add real slots' energies into the per-molecule output [B].

Self-contained: hardcodes problem shapes B=32, N=512, D=384, E=4, H=256.
"""

import os

import numpy as np

import concourse.bass as bass  # noqa: F401  (bass types referenced via bacc/mybir)
import concourse.mybir as mybir
from concourse import bacc
from concourse.bass_utils import run_bass_kernel_spmd
from concourse.tile import TileContext

LOG2 = np.float32(np.log(2.0))
B, N, D = 32, 512, 384
E = 4
H1 = H2 = 256
N_CORES = 8
NT = 512  # moving-operand (slot) tile for matmuls; one PSUM bank at f32

F32 = mybir.dt.float32

# Set by test harnesses: PROFILE=True makes kernel() run with NTFF tracing and
# store the profiled NEFF exec time (ns) in LAST_EXEC_NS.
PROFILE = False
TRACE_CORES = [0]
LAST_EXEC_NS = None

_CACHE: dict = {}


def _build(S: int):
    """Build the per-core Bass graph for S slots (one expert per core)."""
    nc = bacc.Bacc(None, target_bir_lowering=False)

    x_ext = nc.declare_dram_parameter("x", [D, S], F32, isOutput=False)
    w1_ext = nc.declare_dram_parameter("w1", [128, 6 * 128], F32, isOutput=False)
    w2_ext = nc.declare_dram_parameter("w2", [128, 4 * 128], F32, isOutput=False)
    w3_ext = nc.declare_dram_parameter("w3", [128, 2], F32, isOutput=False)
    b1_ext = nc.declare_dram_parameter("b1", [128, 2], F32, isOutput=False)
    b2_ext = nc.declare_dram_parameter("b2", [128, 2], F32, isOutput=False)
    b3_ext = nc.declare_dram_parameter("b3", [1, 1], F32, isOutput=False)
    out_ext = nc.declare_dram_parameter("out", [1, S], F32, isOutput=True)

    # softplus(x) = Ln(Exp(x) + 1): both funcs live in the
    # natural_log_exp_and_others ACT table set (no native softplus table).
    EXP = mybir.ActivationFunctionType.Exp
    LN = mybir.ActivationFunctionType.Ln
    ID = mybir.ActivationFunctionType.Identity

    with TileContext(nc) as tc:
        with (
            tc.tile_pool(name="w", bufs=1) as wp,
            tc.tile_pool(name="xin", bufs=1) as xp,
            tc.tile_pool(name="act", bufs=2) as actp,
            tc.tile_pool(name="ps1", bufs=2, space="PSUM") as pp1,
            tc.tile_pool(name="ps2", bufs=1, space="PSUM") as pp2,
            tc.tile_pool(name="pse", bufs=2, space="PSUM") as ppe,
            tc.tile_pool(name="o", bufs=1) as op,
        ):
            w1 = wp.tile([128, 6 * 128], F32)
            nc.sync.dma_start(w1[:], w1_ext[:])
            w2 = wp.tile([128, 4 * 128], F32)
            nc.sync.dma_start(w2[:], w2_ext[:])
            w3 = wp.tile([128, 2], F32)
            nc.sync.dma_start(w3[:], w3_ext[:])
            b1 = wp.tile([128, 2], F32)
            nc.sync.dma_start(b1[:], b1_ext[:])
            b2 = wp.tile([128, 2], F32)
            nc.sync.dma_start(b2[:], b2_ext[:])
            b3 = wp.tile([1, 1], F32)
            nc.sync.dma_start(b3[:], b3_ext[:])

            out_sb = op.tile([1, S], F32)

            # x in SBUF: 3 d-chunks of [128, S] side by side
            xt = xp.tile([128, 3 * S], F32)
            for d in range(3):
                nc.sync.dma_start(xt[:, d * S : (d + 1) * S], x_ext[d * 128 : (d + 1) * 128, :])

            for t in range(S // NT):
                ts = slice(t * NT, (t + 1) * NT)
                # ---- layer 1: z1[h, s] = sum_d W1[d, h] x[d, s]
                z1 = pp1.tile([128, 2 * NT], F32, tag="z1")
                for h in range(2):
                    for d in range(3):
                        nc.tensor.matmul(
                            z1[:, h * NT : (h + 1) * NT],
                            w1[:, (d * 2 + h) * 128 : (d * 2 + h + 1) * 128],
                            xt[:, d * S + t * NT : d * S + t * NT + NT],
                            start=(d == 0),
                            stop=(d == 2),
                        )
                t1 = actp.tile([128, 2 * NT], F32, tag="t1")
                for h in range(2):
                    nc.scalar.activation(
                        t1[:, h * NT : (h + 1) * NT], z1[:, h * NT : (h + 1) * NT], EXP, bias=b1[:, h : h + 1]
                    )
                a1 = actp.tile([128, 2 * NT], F32, tag="a1")
                nc.scalar.activation(a1[:], t1[:], LN, bias=1.0)
                # ---- layer 2: z2[k, s] = sum_h W2[h, k] a1[h, s]
                z2 = pp2.tile([128, 2 * NT], F32, tag="z2")
                for k in range(2):
                    for h in range(2):
                        nc.tensor.matmul(
                            z2[:, k * NT : (k + 1) * NT],
                            w2[:, (h * 2 + k) * 128 : (h * 2 + k + 1) * 128],
                            a1[:, h * NT : (h + 1) * NT],
                            start=(h == 0),
                            stop=(h == 1),
                        )
                t2 = actp.tile([128, 2 * NT], F32, tag="t2")
                for k in range(2):
                    nc.scalar.activation(
                        t2[:, k * NT : (k + 1) * NT], z2[:, k * NT : (k + 1) * NT], EXP, bias=b2[:, k : k + 1]
                    )
                a2 = actp.tile([128, 2 * NT], F32, tag="a2")
                nc.scalar.activation(a2[:], t2[:], LN, bias=1.0)
                # ---- layer 3: e[s] = sum_k W3[k] a2[k, s]  (+b3 on the copy out)
                er = ppe.tile([1, NT], F32, tag="er")
                for k in range(2):
                    nc.tensor.matmul(
                        er[:],
                        w3[:, k : k + 1],
                        a2[:, k * NT : (k + 1) * NT],
                        start=(k == 0),
                        stop=(k == 1),
                    )
                nc.scalar.activation(out_sb[:, ts], er[:], ID, bias=b3[:])

            nc.sync.dma_start(out_ext[:], out_sb[:])

    nc.finalize()
    return nc


def kernel(representation, atomic_numbers, elements, W1, b1, W2, b2, W3, b3):
    global LAST_EXEC_NS
    rep = np.asarray(representation, dtype=np.float32)
    an = np.asarray(atomic_numbers).astype(np.int64)
    el = np.asarray(elements).astype(np.int64)
    W1 = np.asarray(W1, dtype=np.float32)
    b1 = np.asarray(b1, dtype=np.float32)
    W2 = np.asarray(W2, dtype=np.float32)
    b2 = np.asarray(b2, dtype=np.float32)
    W3 = np.asarray(W3, dtype=np.float32)
    b3 = np.asarray(b3, dtype=np.float32)

    Bsz, Nn, Dd = rep.shape
    flat = rep.reshape(-1, Dd)
    anf = an.reshape(-1)

    idxs = [np.nonzero(anf == el[e])[0] for e in range(E)]
    counts = [len(ix) for ix in idxs]

    # slots per core; expert capacity = 2*S (two cores per expert)
    S = 1024
    while max(counts) > 2 * S:
        S *= 2

    # fold the shifted-softplus -log(2) into downstream biases
    b2_eff = b2 - LOG2 * W2.sum(axis=1)  # [E, H2]
    b3_eff = b3 - LOG2 * W3.sum(axis=1)  # [E]

    if S not in _CACHE:
        _CACHE[S] = _build(S)
    nc = _CACHE[S]

    in_maps = []
    for c in range(N_CORES):
        e, half = divmod(c, 2)
        ix = idxs[e]
        lo = half * S
        hi = min(len(ix), lo + S)
        xs = np.zeros((S, Dd), np.float32)
        if hi > lo:
            xs[: hi - lo] = flat[ix[lo:hi]]
        in_maps.append(
            {
                "x": np.ascontiguousarray(xs.T),
                "w1": np.ascontiguousarray(
                    W1[e].reshape(3, 128, 2, 128).transpose(1, 0, 2, 3).reshape(128, 768)
                ),
                "w2": np.ascontiguousarray(
                    W2[e].reshape(2, 128, 2, 128).transpose(1, 0, 2, 3).reshape(128, 512)
                ),
                "w3": np.ascontiguousarray(W3[e].reshape(2, 128).T),
                "b1": np.ascontiguousarray(b1[e].reshape(2, 128).T),
                "b2": np.ascontiguousarray(b2_eff[e].reshape(2, 128).T),
                "b3": b3_eff[e].reshape(1, 1).astype(np.float32),
            }
        )

    kwargs = {}
    if PROFILE:
        kwargs = dict(trace=True, trace_cores=list(TRACE_CORES))
    res = run_bass_kernel_spmd(nc, in_maps, core_ids=list(range(N_CORES)), **kwargs)
    LAST_EXEC_NS = res.exec_time_ns

    energies = np.zeros(Bsz, np.float64)
    for c in range(N_CORES):
        e, half = divmod(c, 2)
        ix = idxs[e]
        lo = half * S
        hi = min(len(ix), lo + S)
        if hi <= lo:
            continue
        evals = np.asarray(res.results[c]["out"]).reshape(-1)[: hi - lo]
        np.add.at(energies, ix[lo:hi] // Nn, evals.astype(np.float64))
    return energies.astype(np.float32)


# revision 12
# speedup vs baseline: 1.8155x; 1.8155x over previous
"""ANI-style element-MLP (MoE routing) kernel for 8 TRN2 NeuronCores.

Strategy:
  - Host: bucket atoms by element (expert). Only ~4/9 of atoms match any
    expert; the rest contribute 0.  Each expert bucket is padded to a fixed
    capacity, split in half, and each half is assigned to one core
    (cores 2e, 2e+1 own expert e).  Per-core inputs are the gathered,
    transposed representation rows [D, S] plus that expert's weights laid
    out in SBUF-ready [128, ...] chunk order.
  - Device: 3-layer MLP as tiled matmuls (features on partitions so biases
    are per-partition ACT bias), softplus on the scalar engine.  The
    softplus -log(2) shift is folded into the next layer's bias on host.
    Output is the per-slot scalar energy [1, S] per core.
  - Host: scatter-add real slots' energies into the per-molecule output [B].

Self-contained: hardcodes problem shapes B=32, N=512, D=384, E=4, H=256.
"""

import os

import ml_dtypes
import numpy as np

import concourse.bass as bass  # noqa: F401  (bass types referenced via bacc/mybir)
import concourse.mybir as mybir
from concourse import bacc
from concourse.bass_utils import run_bass_kernel_spmd
from concourse.hw_specs import get_activation_tables
from concourse.tile import TileContext


class _OneActSetBacc(bacc.Bacc):
    """All our ACT functions (Exp, Ln, Identity) live in the
    natural_log_exp_and_others table set, but the stock table-load pass
    assigns each function its first matching set, thrashing ~1.5us table
    loads between sets on every layer.  Force every load to the one set
    that covers all three and drop the now-redundant reloads."""

    _ACT_SET = "natural_log_exp_and_others"

    def insert_act_table_loads(self):
        super().insert_act_table_loads()
        names = list(get_activation_tables(self.m.arch))
        target = names.index(self._ACT_SET)
        for blk in self.main_func.blocks:
            seen_engines = set()
            to_remove = []
            for inst in blk.instructions:
                if isinstance(inst, mybir.InstLoadActFuncSet):
                    if inst.engine in seen_engines and not (inst.has_wait() or inst.has_update()):
                        to_remove.append(inst)
                    else:
                        inst.act_func_set_id = target
                        seen_engines.add(inst.engine)
            for inst in to_remove:
                blk.instructions.remove(inst)

LOG2 = np.float32(np.log(2.0))
B, N, D = 32, 512, 384
E = 4
H1 = H2 = 256
N_CORES = 8
NT = 512  # moving-operand (slot) tile for matmuls; one PSUM bank at f32

F32 = mybir.dt.float32

# Set by test harnesses: PROFILE=True makes kernel() run with NTFF tracing and
# store the profiled NEFF exec time (ns) in LAST_EXEC_NS.
PROFILE = False
TRACE_CORES = [0]
LAST_EXEC_NS = None

_CACHE: dict = {}


BF16 = mybir.dt.bfloat16


def _build(S: int):
    """Build the per-core Bass graph for S slots (one expert per core)."""
    nc = _OneActSetBacc(None, target_bir_lowering=False)

    x_ext = nc.declare_dram_parameter("x", [D, S], BF16, isOutput=False)
    w1_ext = nc.declare_dram_parameter("w1", [128, 6 * 128], BF16, isOutput=False)
    w2_ext = nc.declare_dram_parameter("w2", [128, 4 * 128], BF16, isOutput=False)
    w3_ext = nc.declare_dram_parameter("w3", [128, 2], BF16, isOutput=False)
    b1_ext = nc.declare_dram_parameter("b1", [128, 2], F32, isOutput=False)
    b2_ext = nc.declare_dram_parameter("b2", [128, 2], F32, isOutput=False)
    b3_ext = nc.declare_dram_parameter("b3", [1, 1], F32, isOutput=False)
    out_ext = nc.declare_dram_parameter("out", [1, S], F32, isOutput=True)

    # softplus(x) = Ln(Exp(x) + 1): both funcs live in the
    # natural_log_exp_and_others ACT table set (no native softplus table).
    EXP = mybir.ActivationFunctionType.Exp
    LN = mybir.ActivationFunctionType.Ln
    ID = mybir.ActivationFunctionType.Identity

    with TileContext(nc) as tc:
        with (
            tc.tile_pool(name="w", bufs=1) as wp,
            tc.tile_pool(name="xin", bufs=1) as xp,
            tc.tile_pool(name="act", bufs=2) as actp,
            tc.tile_pool(name="ps1", bufs=2, space="PSUM") as pp1,
            tc.tile_pool(name="ps2", bufs=1, space="PSUM") as pp2,
            tc.tile_pool(name="pse", bufs=2, space="PSUM") as ppe,
            tc.tile_pool(name="o", bufs=1) as op,
        ):
            w1 = wp.tile([128, 6 * 128], BF16)
            nc.sync.dma_start(w1[:], w1_ext[:])
            w2 = wp.tile([128, 4 * 128], BF16)
            nc.sync.dma_start(w2[:], w2_ext[:])
            w3 = wp.tile([128, 2], BF16)
            nc.sync.dma_start(w3[:], w3_ext[:])
            b1 = wp.tile([128, 2], F32)
            nc.sync.dma_start(b1[:], b1_ext[:])
            b2 = wp.tile([128, 2], F32)
            nc.sync.dma_start(b2[:], b2_ext[:])
            b3 = wp.tile([1, 1], F32)
            nc.sync.dma_start(b3[:], b3_ext[:])

            out_sb = op.tile([1, S], F32)

            # x in SBUF: one tile per (d-chunk, slot-chunk) so slot-chunk 0's
            # matmuls can start before the whole x DMA lands
            xt = {}
            for t in range(S // NT):
                for d in range(3):
                    xt[d, t] = xp.tile([128, NT], BF16, tag=f"x{d}_{t}", name=f"x{d}_{t}")
                    nc.sync.dma_start(
                        xt[d, t][:], x_ext[d * 128 : (d + 1) * 128, t * NT : (t + 1) * NT]
                    )

            for t in range(S // NT):
                ts = slice(t * NT, (t + 1) * NT)
                # ---- layer 1: z1[h, s] = sum_d W1[d, h] x[d, s]
                z1 = pp1.tile([128, 2 * NT], F32, tag="z1")
                for h in range(2):
                    for d in range(3):
                        nc.tensor.matmul(
                            z1[:, h * NT : (h + 1) * NT],
                            w1[:, (d * 2 + h) * 128 : (d * 2 + h + 1) * 128],
                            xt[d, t][:],
                            start=(d == 0),
                            stop=(d == 2),
                        )
                t1 = actp.tile([128, 2 * NT], F32, tag="t1")
                for h in range(2):
                    nc.scalar.activation(
                        t1[:, h * NT : (h + 1) * NT], z1[:, h * NT : (h + 1) * NT], EXP, bias=b1[:, h : h + 1]
                    )
                a1 = actp.tile([128, 2 * NT], BF16, tag="a1")
                nc.scalar.activation(a1[:], t1[:], LN, bias=1.0)
                # ---- layer 2: z2[k, s] = sum_h W2[h, k] a1[h, s]
                z2 = pp2.tile([128, 2 * NT], F32, tag="z2")
                for k in range(2):
                    for h in range(2):
                        nc.tensor.matmul(
                            z2[:, k * NT : (k + 1) * NT],
                            w2[:, (h * 2 + k) * 128 : (h * 2 + k + 1) * 128],
                            a1[:, h * NT : (h + 1) * NT],
                            start=(h == 0),
                            stop=(h == 1),
                        )
                t2 = actp.tile([128, 2 * NT], F32, tag="t2")
                for k in range(2):
                    nc.scalar.activation(
                        t2[:, k * NT : (k + 1) * NT], z2[:, k * NT : (k + 1) * NT], EXP, bias=b2[:, k : k + 1]
                    )
                a2 = actp.tile([128, 2 * NT], BF16, tag="a2")
                nc.scalar.activation(a2[:], t2[:], LN, bias=1.0)
                # ---- layer 3: e[s] = sum_k W3[k] a2[k, s]  (+b3 on the copy out)
                er = ppe.tile([1, NT], F32, tag="er")
                for k in range(2):
                    nc.tensor.matmul(
                        er[:],
                        w3[:, k : k + 1],
                        a2[:, k * NT : (k + 1) * NT],
                        start=(k == 0),
                        stop=(k == 1),
                    )
                nc.scalar.activation(out_sb[:, ts], er[:], ID, bias=b3[:])

            nc.sync.dma_start(out_ext[:], out_sb[:])

    nc.finalize()
    return nc


def kernel(representation, atomic_numbers, elements, W1, b1, W2, b2, W3, b3):
    global LAST_EXEC_NS
    rep = np.asarray(representation, dtype=np.float32)
    an = np.asarray(atomic_numbers).astype(np.int64)
    el = np.asarray(elements).astype(np.int64)
    W1 = np.asarray(W1, dtype=np.float32)
    b1 = np.asarray(b1, dtype=np.float32)
    W2 = np.asarray(W2, dtype=np.float32)
    b2 = np.asarray(b2, dtype=np.float32)
    W3 = np.asarray(W3, dtype=np.float32)
    b3 = np.asarray(b3, dtype=np.float32)

    Bsz, Nn, Dd = rep.shape
    flat = rep.reshape(-1, Dd)
    anf = an.reshape(-1)

    idxs = [np.nonzero(anf == el[e])[0] for e in range(E)]
    counts = [len(ix) for ix in idxs]

    # slots per core; expert capacity = 2*S (two cores per expert)
    S = 1024
    while max(counts) > 2 * S:
        S *= 2

    # fold the shifted-softplus -log(2) into downstream biases
    b2_eff = b2 - LOG2 * W2.sum(axis=1)  # [E, H2]
    b3_eff = b3 - LOG2 * W3.sum(axis=1)  # [E]

    if S not in _CACHE:
        _CACHE[S] = _build(S)
    nc = _CACHE[S]

    in_maps = []
    for c in range(N_CORES):
        e, half = divmod(c, 2)
        ix = idxs[e]
        lo = half * S
        hi = min(len(ix), lo + S)
        bf16 = ml_dtypes.bfloat16
        xs = np.zeros((S, Dd), np.float32)
        if hi > lo:
            xs[: hi - lo] = flat[ix[lo:hi]]
        in_maps.append(
            {
                "x": np.ascontiguousarray(xs.T).astype(bf16),
                "w1": np.ascontiguousarray(
                    W1[e].reshape(3, 128, 2, 128).transpose(1, 0, 2, 3).reshape(128, 768)
                ).astype(bf16),
                "w2": np.ascontiguousarray(
                    W2[e].reshape(2, 128, 2, 128).transpose(1, 0, 2, 3).reshape(128, 512)
                ).astype(bf16),
                "w3": np.ascontiguousarray(W3[e].reshape(2, 128).T).astype(bf16),
                "b1": np.ascontiguousarray(b1[e].reshape(2, 128).T),
                "b2": np.ascontiguousarray(b2_eff[e].reshape(2, 128).T),
                "b3": b3_eff[e].reshape(1, 1).astype(np.float32),
            }
        )

    kwargs = {}
    if PROFILE:
        kwargs = dict(trace=True, trace_cores=list(TRACE_CORES))
    res = run_bass_kernel_spmd(nc, in_maps, core_ids=list(range(N_CORES)), **kwargs)
    LAST_EXEC_NS = res.exec_time_ns

    energies = np.zeros(Bsz, np.float64)
    for c in range(N_CORES):
        e, half = divmod(c, 2)
        ix = idxs[e]
        lo = half * S
        hi = min(len(ix), lo + S)
        if hi <= lo:
            continue
        evals = np.asarray(res.results[c]["out"]).reshape(-1)[: hi - lo]
        np.add.at(energies, ix[lo:hi] // Nn, evals.astype(np.float64))
    return energies.astype(np.float32)


# revision 23
# speedup vs baseline: 2.0087x; 1.1064x over previous
"""ANI-style element-MLP (MoE routing) kernel for 8 TRN2 NeuronCores.

Strategy:
  - Host: bucket atoms by element (expert). Only ~4/9 of atoms match any
    expert; the rest contribute 0.  Each expert bucket is padded to a fixed
    capacity, split in half, and each half is assigned to one core
    (cores 2e, 2e+1 own expert e).  Per-core inputs are the gathered,
    transposed representation rows [D, S] plus that expert's weights laid
    out in SBUF-ready [128, ...] chunk order.
  - Device: 3-layer MLP as tiled matmuls (features on partitions so biases
    are per-partition ACT bias), softplus on the scalar engine.  The
    softplus -log(2) shift is folded into the next layer's bias on host.
    Output is the per-slot scalar energy [1, S] per core.
  - Host: scatter-add real slots' energies into the per-molecule output [B].

Self-contained: hardcodes problem shapes B=32, N=512, D=384, E=4, H=256.
"""

import os

import ml_dtypes
import numpy as np

import concourse.bass as bass  # noqa: F401  (bass types referenced via bacc/mybir)
import concourse.mybir as mybir
from concourse import bacc
from concourse.bass_utils import run_bass_kernel_spmd
from concourse.hw_specs import get_activation_tables
from concourse.tile import TileContext


class _OneActSetBacc(bacc.Bacc):
    """All our ACT functions (Exp, Ln, Identity) live in the
    natural_log_exp_and_others table set, but the stock table-load pass
    assigns each function its first matching set, thrashing ~1.5us table
    loads between sets on every layer.  Force every load to the one set
    that covers all three and drop the now-redundant reloads."""

    _ACT_SET = "natural_log_exp_and_others"

    def insert_act_table_loads(self):
        super().insert_act_table_loads()
        names = list(get_activation_tables(self.m.arch))
        target = names.index(self._ACT_SET)
        for blk in self.main_func.blocks:
            seen_engines = set()
            to_remove = []
            for inst in blk.instructions:
                if isinstance(inst, mybir.InstLoadActFuncSet):
                    if inst.engine in seen_engines and not (inst.has_wait() or inst.has_update()):
                        to_remove.append(inst)
                    else:
                        inst.act_func_set_id = target
                        seen_engines.add(inst.engine)
            for inst in to_remove:
                blk.instructions.remove(inst)

LOG2 = np.float32(np.log(2.0))
B, N, D = 32, 512, 384
E = 4
H1 = H2 = 256
N_CORES = 8
NT = 512  # moving-operand (slot) tile for matmuls; one PSUM bank at f32

F32 = mybir.dt.float32

# Set by test harnesses: PROFILE=True makes kernel() run with NTFF tracing and
# store the profiled NEFF exec time (ns) in LAST_EXEC_NS.
PROFILE = False
TRACE_CORES = [0]
LAST_EXEC_NS = None

_CACHE: dict = {}


BF16 = mybir.dt.bfloat16


def _build(S: int):
    """Raw-Bass per-core graph for S slots (one expert per core).

    Engine plan (explicit semaphores, no Tile):
      sync   : x DMAs in, final out DMA
      scalar : weight/bias DMAs (2nd HWDGE queue), all Exp/Ln activations
      tensor : all matmuls (z1/z2 per slot-chunk + the W3 row, PSUM-aliased)
      vector : +b3 epilogue copy PSUM->SBUF out
    """
    from contextlib import ExitStack

    nc = _OneActSetBacc(None, target_bir_lowering=False)

    x_ext = nc.declare_dram_parameter("x", [128, 3 * S], BF16, isOutput=False)
    wt_ext = nc.declare_dram_parameter("wt", [128, 1282], BF16, isOutput=False)
    bias_ext = nc.declare_dram_parameter("bias", [128, 5], F32, isOutput=False)
    out_ext = nc.declare_dram_parameter("out", [1, S], F32, isOutput=True)

    EXP = mybir.ActivationFunctionType.Exp
    LN = mybir.ActivationFunctionType.Ln
    ID = mybir.ActivationFunctionType.Identity

    TCH = S // NT  # slot chunks (2 for S=1024)
    assert TCH == 2, "sem schedule below is written for 2 slot chunks"

    with ExitStack() as ctx:
        xt = ctx.enter_context(nc.sbuf_tensor([128, 3 * S], BF16))
        wt = ctx.enter_context(nc.sbuf_tensor([128, 1282], BF16))
        bias = ctx.enter_context(nc.sbuf_tensor([128, 5], F32))
        scratch = ctx.enter_context(nc.sbuf_tensor([1, 16], F32))
        out_sb = ctx.enter_context(nc.sbuf_tensor([1, S], F32))
        t1 = [ctx.enter_context(nc.sbuf_tensor(f"t1_{t}", [128, 2 * NT], F32)) for t in range(TCH)]
        a1 = [ctx.enter_context(nc.sbuf_tensor(f"a1_{t}", [128, 2 * NT], BF16)) for t in range(TCH)]
        t2 = [ctx.enter_context(nc.sbuf_tensor(f"t2_{t}", [128, 2 * NT], F32)) for t in range(TCH)]
        a2 = [ctx.enter_context(nc.sbuf_tensor(f"a2_{t}", [128, 2 * NT], BF16)) for t in range(TCH)]
        z1 = [ctx.enter_context(nc.psum_tensor(f"z1_{t}", [128, 2 * NT], F32)) for t in range(TCH)]
        z2 = [ctx.enter_context(nc.psum_tensor(f"z2_{t}", [128, 2 * NT], F32)) for t in range(TCH)]
        sem_x0 = ctx.enter_context(nc.semaphore("sem_x0"))
        sem_x1 = ctx.enter_context(nc.semaphore("sem_x1"))
        sem_w = ctx.enter_context(nc.semaphore("sem_w"))
        sem_b = ctx.enter_context(nc.semaphore("sem_b"))
        sem_o = ctx.enter_context(nc.semaphore("sem_o"))
        sem_mm = ctx.enter_context(nc.semaphore("sem_mm"))
        sem_act = ctx.enter_context(nc.semaphore("sem_act"))
        sem_v = ctx.enter_context(nc.semaphore("sem_v"))
        block = ctx.enter_context(nc.Block())

        # the W3 energy row reuses z2[t]'s first bank, partition 0 (its
        # matmuls run only after the Exps have drained z2[t])
        er = [z2[t][0:1, 0:NT] for t in range(TCH)]

        def w1s(d, h):
            return wt[:, (d * 2 + h) * 128 : (d * 2 + h + 1) * 128]

        def w2s(h, k):
            return wt[:, 768 + (h * 2 + k) * 128 : 768 + (h * 2 + k + 1) * 128]

        def w3s(k):
            return wt[:, 1280 + k : 1281 + k]

        @block.sync
        def _(sync):
            # host supplies x pre-laid-out as [128, t*(3*NT) + d*NT + s]
            for t, sem in enumerate([sem_x0, sem_x1]):
                sync.dma_start(
                    xt[:, t * 3 * NT : (t + 1) * 3 * NT],
                    x_ext[:, t * 3 * NT : (t + 1) * 3 * NT],
                ).then_inc(sem, 16)
            sync.wait_ge(sem_v, TCH)
            sync.dma_start(out_ext[:], out_sb[:]).then_inc(sem_o, 16)
            sync.wait_ge(sem_o, 16)

        @block.scalar
        def _(scalar):
            scalar.dma_start(wt[:], wt_ext[:]).then_inc(sem_w, 16)
            scalar.dma_start(bias[:], bias_ext[:]).then_inc(sem_b, 16)
            # memzero lowers to an ACTIVATE, anchoring the ACT table load
            # before any cross-engine waits
            scalar.memzero(scratch[:])
            scalar.wait_ge(sem_b, 16)
            # PE sem_mm cumulative: l1(0)=1, l1(1)=2, l2(0)=3, l3(0)=4,
            # l2(1)=5, l3(1)=6
            for t in range(TCH):
                scalar.wait_ge(sem_mm, [1, 2][t])
                for h in range(2):
                    scalar.activation(
                        t1[t][:, h * NT : (h + 1) * NT],
                        z1[t][:, h * NT : (h + 1) * NT],
                        EXP,
                        bias=bias[:, h : h + 1],
                    ).then_inc(sem_act, 1)
                scalar.wait_ge(sem_act, 6 * t + 2)  # ACT pipeline RAW: t1 fully written
                scalar.activation(a1[t][:], t1[t][:], LN, bias=1.0).then_inc(sem_act, 1)
                scalar.wait_ge(sem_mm, [3, 5][t])
                for k in range(2):
                    scalar.activation(
                        t2[t][:, k * NT : (k + 1) * NT],
                        z2[t][:, k * NT : (k + 1) * NT],
                        EXP,
                        bias=bias[:, 2 + k : 3 + k],
                    ).then_inc(sem_act, 1)
                scalar.wait_ge(sem_act, 6 * t + 5)  # ACT pipeline RAW: t2 fully written
                scalar.activation(a2[t][:], t2[t][:], LN, bias=1.0).then_inc(sem_act, 1)

        @block.tensor
        def _(tensor):
            def l1(t, inc):
                for h in range(2):
                    for d in range(3):
                        mm = tensor.matmul(
                            z1[t][:, h * NT : (h + 1) * NT],
                            w1s(d, h),
                            xt[:, (t * 3 + d) * NT : (t * 3 + d + 1) * NT],
                            start=(d == 0),
                            stop=(d == 2),
                        )
                mm.then_inc(sem_mm, 1)

            def l2(t):
                for k in range(2):
                    for h in range(2):
                        mm = tensor.matmul(
                            z2[t][:, k * NT : (k + 1) * NT],
                            w2s(h, k),
                            a1[t][:, h * NT : (h + 1) * NT],
                            start=(h == 0),
                            stop=(h == 1),
                        )
                mm.then_inc(sem_mm, 1)

            def l3(t):
                for k in range(2):
                    mm = tensor.matmul(
                        er[t],
                        w3s(k),
                        a2[t][:, k * NT : (k + 1) * NT],
                        start=(k == 0),
                        stop=(k == 1),
                        skip_group_check=True,
                    )
                mm.then_inc(sem_mm, 1)

            tensor.wait_ge(sem_w, 16)
            tensor.wait_ge(sem_x0, 16)
            l1(0, 1)
            tensor.wait_ge(sem_x1, 16)
            l1(1, 2)
            tensor.wait_ge(sem_act, 3)
            l2(0)  # sem_mm -> 3
            tensor.wait_ge(sem_act, 6)
            l3(0)  # sem_mm -> 4
            tensor.wait_ge(sem_act, 9)
            l2(1)  # sem_mm -> 5
            tensor.wait_ge(sem_act, 12)
            l3(1)  # sem_mm -> 6

        @block.vector
        def _(vector):
            for t in range(TCH):
                vector.wait_ge(sem_mm, 4 + 2 * t)
                vector.tensor_scalar_add(
                    out_sb[:, t * NT : (t + 1) * NT], er[t], bias[0:1, 4:5]
                ).then_inc(sem_v, 1)

    nc.finalize()
    return nc


def kernel(representation, atomic_numbers, elements, W1, b1, W2, b2, W3, b3):
    global LAST_EXEC_NS
    rep = np.asarray(representation, dtype=np.float32)
    an = np.asarray(atomic_numbers).astype(np.int64)
    el = np.asarray(elements).astype(np.int64)
    W1 = np.asarray(W1, dtype=np.float32)
    b1 = np.asarray(b1, dtype=np.float32)
    W2 = np.asarray(W2, dtype=np.float32)
    b2 = np.asarray(b2, dtype=np.float32)
    W3 = np.asarray(W3, dtype=np.float32)
    b3 = np.asarray(b3, dtype=np.float32)

    Bsz, Nn, Dd = rep.shape
    flat = rep.reshape(-1, Dd)
    anf = an.reshape(-1)

    idxs = [np.nonzero(anf == el[e])[0] for e in range(E)]
    counts = [len(ix) for ix in idxs]

    # slots per core; expert capacity = 2*S (two cores per expert)
    S = 1024
    while max(counts) > 2 * S:
        S *= 2

    # fold the shifted-softplus -log(2) into downstream biases
    b2_eff = b2 - LOG2 * W2.sum(axis=1)  # [E, H2]
    b3_eff = b3 - LOG2 * W3.sum(axis=1)  # [E]

    if S not in _CACHE:
        _CACHE[S] = _build(S)
    nc = _CACHE[S]

    in_maps = []
    for c in range(N_CORES):
        e, half = divmod(c, 2)
        ix = idxs[e]
        lo = half * S
        hi = min(len(ix), lo + S)
        bf16 = ml_dtypes.bfloat16
        xs = np.zeros((S, Dd), np.float32)
        if hi > lo:
            xs[: hi - lo] = flat[ix[lo:hi]]
        wt = np.zeros((128, 1282), np.float32)
        wt[:, 0:768] = W1[e].reshape(3, 128, 2, 128).transpose(1, 0, 2, 3).reshape(128, 768)
        wt[:, 768:1280] = W2[e].reshape(2, 128, 2, 128).transpose(1, 0, 2, 3).reshape(128, 512)
        wt[:, 1280:1282] = W3[e].reshape(2, 128).T
        bias = np.zeros((128, 5), np.float32)
        bias[:, 0:2] = b1[e].reshape(2, 128).T
        bias[:, 2:4] = b2_eff[e].reshape(2, 128).T
        bias[0, 4] = b3_eff[e]
        in_maps.append(
            {
                "x": np.ascontiguousarray(
                    xs.T.reshape(3, 128, S // NT, NT).transpose(1, 2, 0, 3).reshape(128, 3 * S)
                ).astype(bf16),
                "wt": wt.astype(bf16),
                "bias": bias,
            }
        )

    kwargs = {}
    if PROFILE:
        kwargs = dict(trace=True, trace_cores=list(TRACE_CORES))
    res = run_bass_kernel_spmd(nc, in_maps, core_ids=list(range(N_CORES)), **kwargs)
    LAST_EXEC_NS = res.exec_time_ns

    energies = np.zeros(Bsz, np.float64)
    for c in range(N_CORES):
        e, half = divmod(c, 2)
        ix = idxs[e]
        lo = half * S
        hi = min(len(ix), lo + S)
        if hi <= lo:
            continue
        evals = np.asarray(res.results[c]["out"]).reshape(-1)[: hi - lo]
        np.add.at(energies, ix[lo:hi] // Nn, evals.astype(np.float64))
    return energies.astype(np.float32)
